# revision 40
# baseline (speedup 1.0000x reference)
"""Performer attention (causal, kernelized) — Trainium2 Bass kernel, v5 (bf16).

Two launches on 8 cores:

  A) seq-sharded prep: core j owns 256 sequence positions and computes, for
     ALL 8 heads at once: kh (scaled k-projection), the LayerNorm-folded and
     scaled/biased q-projection qh, the v-projection in seq-major layout,
     and the local stabilizer max(h_k).

  B) head-sharded attention: core h owns head h end-to-end: Performer
     feature maps, the causal chunked prefix scan (per-chunk states, C=128),
     output normalization and its row-block of the FC (W_fc row-sharded;
     host sums partials and adds bias + residual).

All matmul operands are bfloat16 (PE: 1.0 cycles/row at any free size; DMA
bytes halved; PSUM accumulation stays fp32).  Algebra:
  - q LayerNorm folded: Wq_eff = diag(gamma) Wq * scale, bias cq = beta@Wq*scale,
    applied to (q - mu) * rstd with rstd = rsqrt(var + eps).
  - exp(h_q + (proj_q - h_q)) == exp(proj_q): q-side stabilizer cancels.
  - k feature: exp(proj_k + h_k - k_stab) via the augmented contraction
    [kh; kh^2] . [rf^T; -0.5] plus a constant bias of -k_stab in the exp.
  - the reference's +KERNEL_EPS on both kernels perturbs the attention
    output by ~1e-4 relative; the attention term itself is ~1e-5 of the
    output norm (the reference's double normalization divides by an extra
    q'.sum(k') factor ~1e5), so the eps terms are ~1e-9 of the output and
    are omitted (validated well under the 2e-2 gate).
  - causal prefix scan chunked at C=128 with per-chunk states; diagonal
    128x128 score blocks handle intra-chunk causality via a triangular mask.
  - D (causal normalizer) rides in state/score column 64 (v column 64 == 1);
    d = q~ . z (z = column sums of k~) in o column 66.
  - the reference's |d|<=1e-6 guard is dead for any realistic data and is
    omitted.
"""

import sys
for _p in ("/opt/trn_rl_repo", "/root/.axon_site/_ro/trn_rl_repo"):
    if _p not in sys.path:
        sys.path.append(_p)

import numpy as np
import ml_dtypes

import concourse.bass as bass
from concourse import bacc
import concourse.mybir as mybir
import concourse.tile as tile
from concourse.bass import ts, ds
from concourse.bass_utils import run_bass_kernel_spmd

F32 = mybir.dt.float32
BF16 = mybir.dt.bfloat16
NPBF = ml_dtypes.bfloat16
NC = 8
N = 2048
D_MODEL = 512
D_K = 64
D_V = 64
M = 266
C = 128
NCH = N // C            # 16 chunks
SLA = N // NC           # 256 seq positions per phase-A core
NSL = 4                 # 512-wide slices of the full sequence
SL = 512
KERNEL_EPS = 1e-4
LN_EPS = 1e-6
SCALE = float(D_MODEL) ** (-0.25)
EXP = mybir.ActivationFunctionType.Exp
SQRT = mybir.ActivationFunctionType.Sqrt
IDENT = mybir.ActivationFunctionType.Identity


# --------------------------------------------------------------------------
# Phase A: seq-sharded projections + local stabilizer
# --------------------------------------------------------------------------
def build_phase_a():
    nc = bacc.Bacc("TRN2", target_bir_lowering=False, debug=False, num_devices=NC)
    xs = nc.dram_tensor("xs", [D_MODEL, 3 * SLA], BF16, kind="ExternalInput")
    Wke = nc.dram_tensor("Wke", [D_MODEL, D_MODEL], BF16, kind="ExternalInput")
    Wqv = nc.dram_tensor("Wqv", [D_MODEL, 2 * D_MODEL], BF16, kind="ExternalInput")
    # bf16 consts [128, 644]: cols 0:4 = wmean|ones|neghalf-lo|neghalf-hi
    # (all partitions); cols 4:644 = row consts on partition 0 only:
    # -sum(Wqe) (512) | ones (128)
    cba = nc.dram_tensor("cba", [128, 4 + D_MODEL + 128], BF16,
                         kind="ExternalInput")
    # f32 consts [128, 5]: cq (4 cols) | col4 row0 = LN_EPS
    cqm = nc.dram_tensor("cqm", [128, 5], F32, kind="ExternalInput")
    kh_out = nc.dram_tensor("kh", [D_MODEL, SLA], BF16, kind="ExternalOutput")
    qh_out = nc.dram_tensor("qh", [D_MODEL, SLA], BF16, kind="ExternalOutput")
    vhT_out = nc.dram_tensor("vhT", [SLA, D_MODEL], BF16, kind="ExternalOutput")
    stab_out = nc.dram_tensor("stab", [2, 1], F32, kind="ExternalOutput")

    with tile.TileContext(nc) as tc:
        with (
            tc.tile_pool(name="wts", bufs=1) as wts,
            tc.tile_pool(name="xin", bufs=1) as xin,
            tc.tile_pool(name="work", bufs=1) as work,
            tc.tile_pool(name="stat", bufs=1) as statp,
            tc.tile_pool(name="outs", bufs=1) as outs,
        ):
            # ---- loads: consts first (tiny), then Wk+k (kh path), q (LN),
            # v, Wq, Wv
            cqm_sb = wts.tile([128, 5], F32)
            nc.sync.dma_start(out=cqm_sb, in_=cqm[:, :])
            cba_sb = wts.tile([128, 4 + D_MODEL + 128], BF16)
            nc.sync.dma_start(out=cba_sb, in_=cba[:, :])
            cb_sb = cba_sb[:, 0:4]
            gq_sb = cba_sb[0:1, 4:4 + D_MODEL + 128]
            x_r = xin.tile([128, 4, 3 * SLA], BF16)
            nc.sync.dma_start(out=x_r, in_=xs[:, :].rearrange("(c p) f -> p c f", p=128))
            wk_r = wts.tile([128, 4, D_MODEL], BF16)
            nc.sync.dma_start(out=wk_r, in_=Wke[:, :].rearrange("(c p) f -> p c f", p=128))
            q_r = x_r[:, :, 0:SLA]
            k_r = x_r[:, :, SLA:2 * SLA]
            v_r = x_r[:, :, 2 * SLA:3 * SLA]
            wqv_r = wts.tile([128, 4, 2 * D_MODEL], BF16)
            nc.sync.dma_start(out=wqv_r, in_=Wqv[:, :].rearrange("(c p) f -> p c f", p=128))
            wq_r = wqv_r[:, :, 0:D_MODEL]
            wv_r = wqv_r[:, :, D_MODEL:2 * D_MODEL]

            wm_r = cb_sb[:, 0:1]      # 1/512
            nh_r = cb_sb[:, 2:4]      # -0.5 split into per-head halves
            cq_sb = cqm_sb[:, 0:4]
            eps_sb = cqm_sb[0:1, 4:5]

            def q_c(c):
                return q_r[:, c, :]

            def k_c(c):
                return k_r[:, c, :]

            def v_c(c):
                return v_r[:, c, :]

            # ---- LayerNorm stats on q (over d_model, per position) ----
            mu_r = statp.tile([1, SLA], BF16)
            with tc.tile_pool(name="psb", bufs=2, space="PSUM") as psb:
                mu_ps = psb.tile([1, SLA], F32, tag="mu", bufs=1)
                for c in range(4):
                    nc.tensor.matmul(mu_ps, wm_r, q_c(c), start=(c == 0),
                                     stop=(c == 3), skip_group_check=True)
                qsq_r = work.tile([128, 4, SLA], BF16)
                for c in range(4):
                    nc.vector.tensor_mul(qsq_r[:, c, :], q_c(c), q_c(c))
                msq_ps = psb.tile([1, SLA], F32, tag="msq", bufs=1)
                for c in range(4):
                    nc.tensor.matmul(msq_ps, wm_r, qsq_r[:, c, :], start=(c == 0),
                                     stop=(c == 3), skip_group_check=True)
                mu_f = statp.tile([1, SLA], F32)
                nc.vector.tensor_copy(mu_f, mu_ps)
                nc.vector.tensor_copy(mu_r, mu_ps)
                var_sb = statp.tile([1, SLA], F32)
                nc.vector.tensor_mul(var_sb, mu_f, mu_f)
                nc.vector.tensor_sub(var_sb, msq_ps, var_sb)
                srt_sb = statp.tile([1, SLA], F32)
                nc.scalar.activation(srt_sb, var_sb, SQRT,
                                     bias=eps_sb, scale=1.0)
                rstd_r = statp.tile([1, SLA], BF16)
                with nc.allow_low_precision(reason="bf16 layernorm scale"):
                    nc.vector.reciprocal(rstd_r, srt_sb)

                # ---- projections: kh (only needs Wk), then qh, then vh;
                # same psum pool so no pool-close barrier stalls the PE
                kh_sb = outs.tile([128, 4, SLA], BF16)
                kh2_r = work.tile([128, 4, SLA], BF16)
                qh_sb = outs.tile([128, 4, SLA], BF16)
                vhT_sb = outs.tile([128, 2, D_MODEL], BF16)
                for oc in range(4):
                    kh_ps = psb.tile([128, D_MODEL], F32, tag="khvh",
                                     name=f"kh{oc}")
                    for c in range(4):
                        nc.tensor.matmul(kh_ps[:, 0:SLA], wk_r[:, c, ts(oc, 128)],
                                         k_c(c), start=(c == 0),
                                         stop=(c == 3), skip_group_check=True)
                    nc.scalar.copy(kh_sb[:, oc, :], kh_ps[:, 0:SLA])
                    nc.vector.tensor_mul(kh2_r[:, oc, :], kh_sb[:, oc, :],
                                         kh_sb[:, oc, :])
                nc.sync.dma_start(
                    out=kh_out[:, :].rearrange("(c p) f -> p c f", p=128),
                    in_=kh_sb)

                # local stabilizer max over heads/positions of -0.5||kh_h||^2;
                # each 128-partition oc chunk holds 2 heads (64 dims each)
                hkm = statp.tile([2, 4], F32)
                for oc in range(4):
                    hk_ps = psb.tile([2, SLA], F32, tag="hk", bufs=1,
                                     name=f"hk{oc}")
                    nc.tensor.matmul(hk_ps, nh_r, kh2_r[:, oc, :],
                                     start=True, stop=True,
                                     skip_group_check=True)
                    nc.vector.reduce_max(hkm[:, oc:oc + 1], hk_ps,
                                         axis=mybir.AxisListType.X)
                stab_sb = statp.tile([2, 1], F32)
                nc.vector.reduce_max(stab_sb, hkm, axis=mybir.AxisListType.X)
                nc.scalar.dma_start(out=stab_out[:, :], in_=stab_sb)

                # rstd broadcast to 128 partitions
                rsbc_sb = work.tile([128, SLA], BF16)
                rsbc_ps = psb.tile([128, SLA], F32, tag="rsbc", bufs=1)
                nc.tensor.matmul(rsbc_ps, gq_sb[0:1, D_MODEL:D_MODEL + 128],
                                 rstd_r, start=True, stop=True,
                                 skip_group_check=True)
                nc.scalar.copy(rsbc_sb, rsbc_ps)

                for oc in range(4):
                    qh_ps = psb.tile([128, SLA], F32, tag="qh")
                    for c in range(4):
                        nc.tensor.matmul(qh_ps, wq_r[:, c, ts(oc, 128)],
                                         q_c(c), start=(c == 0),
                                         stop=False, skip_group_check=True)
                    nc.tensor.matmul(qh_ps, gq_sb[0:1, ts(oc, 128)], mu_r,
                                     start=False, stop=True,
                                     skip_group_check=True)
                    nc.vector.tensor_mul(qh_sb[:, oc, :], qh_ps, rsbc_sb)
                    nc.scalar.activation(qh_sb[:, oc, :], qh_sb[:, oc, :], IDENT,
                                         bias=cq_sb[:, oc:oc + 1], scale=1.0)
                nc.sync.dma_start(
                    out=qh_out[:, :].rearrange("(c p) f -> p c f", p=128),
                    in_=qh_sb)

                for sc in range(2):
                    vh_ps = psb.tile([128, D_MODEL], F32, tag="khvh",
                                     name=f"vh{sc}")
                    for c in range(4):
                        nc.tensor.matmul(vh_ps, v_c(c)[:, ts(sc, 128)],
                                         wv_r[:, c, :], start=(c == 0),
                                         stop=(c == 3), skip_group_check=True)
                    if sc == 0:
                        nc.scalar.copy(vhT_sb[:, sc, :], vh_ps)
                    else:
                        nc.vector.tensor_copy(vhT_sb[:, sc, :], vh_ps)
                nc.sync.dma_start(
                    out=vhT_out[:, :].rearrange("(s p) f -> p s f", p=128),
                    in_=vhT_sb)
    nc.compile()
    return nc


# --------------------------------------------------------------------------
# Phase B: head-sharded Performer attention + FC row-block
# --------------------------------------------------------------------------
def build_phase_b(debug=False):
    nc = bacc.Bacc("TRN2", target_bir_lowering=False, debug=False, num_devices=NC)
    khh = nc.dram_tensor("khh", [D_K, N], BF16, kind="ExternalInput")
    qhh = nc.dram_tensor("qhh", [D_K, N], BF16, kind="ExternalInput")
    vht = nc.dram_tensor("vht", [128, NCH * D_V], BF16, kind="ExternalInput")
    # bf16 const blobs: blob1 = rft|rneg (needed first), blob2 = idm|tri|wfc
    NB1 = 2 * M
    NB2 = 128 + 128 + D_MODEL
    blob1 = nc.dram_tensor("blob1", [D_K, NB1], BF16, kind="ExternalInput")
    blob2 = nc.dram_tensor("blob2", [128, NB2], BF16, kind="ExternalInput")
    # f32 consts [128, 129]: col0 = -k_stab, cols 1:129 identity (transpose)
    stabc = nc.dram_tensor("stabc", [128, 129], F32, kind="ExternalInput")
    out_d = nc.dram_tensor("out", [N, D_MODEL], BF16, kind="ExternalOutput")

    with tile.TileContext(nc) as tc:
        with (
            tc.tile_pool(name="consts", bufs=1) as consts,
            tc.tile_pool(name="krows", bufs=1) as krows,
            tc.tile_pool(name="feat", bufs=1) as feat,
            tc.tile_pool(name="ktrp", bufs=1) as ktrp,
            tc.tile_pool(name="ktT", bufs=NCH) as ktTp,
            tc.tile_pool(name="atp", bufs=NCH) as atp,
            tc.tile_pool(name="ssb", bufs=NCH + 1) as ssbp,
            tc.tile_pool(name="post", bufs=6) as post,
            tc.tile_pool(name="outp", bufs=4) as outp,
        ):
            # ---- loads (order: what the k/q feature path needs first) ----
            b1_sb = consts.tile([D_K, NB1], BF16)
            nc.scalar.dma_start(out=b1_sb, in_=blob1[:, :])
            stab_full = consts.tile([128, 129], F32)
            nc.scalar.dma_start(out=stab_full, in_=stabc[:, :])
            stab_sb = stab_full[:, 0:1]
            idf_r = stab_full[:, 1:129]
            khr = krows.tile([D_K, N], BF16)
            nc.sync.dma_start(out=khr, in_=khh[:, :])
            qhr = krows.tile([D_K, N], BF16)
            nc.sync.dma_start(out=qhr, in_=qhh[:, :])
            b2_sb = consts.tile([128, NB2], BF16)
            nc.scalar.dma_start(out=b2_sb, in_=blob2[:, :])
            vha = krows.tile([128, NCH, 66], BF16)
            nc.gpsimd.dma_start(
                out=vha[:, :, 0:D_V],
                in_=vht[:, :].rearrange("p (ch f) -> p ch f", ch=NCH))

            rft_r = b1_sb[:, 0:M]
            rneg_r = b1_sb[:, M:2 * M]
            id_r = b2_sb[:, 0:128]
            tri_r = b2_sb[:, 128:256]
            wfc_r = b2_sb[0:D_V, 256:256 + D_MODEL]

            # vha constant columns: 64 -> 1.0, 65 -> 0.0
            nc.gpsimd.memset(vha[:, :, D_V:D_V + 1], 1.0)
            nc.gpsimd.memset(vha[:, :, D_V + 1:D_V + 2], 0.0)

            # kh^2 rows
            kh2r = krows.tile([D_K, N], BF16)
            nc.vector.tensor_mul(kh2r, khr, khr)

            # q~ features m-major [128, 3, N]; mc2 = rows 0..9
            qt_feat = feat.tile([128, 3, N], BF16)
            # k~ features seq-major [C, ch, 266]
            ktr = ktrp.tile([128, NCH, M], BF16)

            s_tiles = [ssbp.tile([128, 3, 66], BF16, tag="ssb", name=f"ssb{i}")
                       for i in range(NCH + 1)]

            ktT_tiles = {}
            at_list = []
            with (
                tc.tile_pool(name="psq", bufs=2, space="PSUM") as psq,
                tc.tile_pool(name="pskp", bufs=2, space="PSUM") as pskp,
                tc.tile_pool(name="pstr", bufs=2, space="PSUM") as pstr,
                tc.tile_pool(name="psat", bufs=1, space="PSUM") as psat,
                tc.tile_pool(name="pssd", bufs=1, space="PSUM") as pssd,
            ):
              def emit_kp(ch):
                    # k-feature projection for chunk ch (prefetched one chunk
                    # ahead so the PE isn't idle while ACT runs the exp)
                    kp_ps = pskp.tile([C, M], F32, tag="kp", name=f"kp{ch}")
                    nc.tensor.matmul(kp_ps, khr[:, ts(ch, C)], rft_r,
                                     start=True, stop=False,
                                     skip_group_check=True)
                    nc.tensor.matmul(kp_ps, kh2r[:, ts(ch, C)], rneg_r,
                                     start=False, stop=True,
                                     skip_group_check=True)
                    with nc.allow_low_precision(reason="bf16 features"):
                        nc.scalar.activation(
                            ktr[:, ch, 0:M], kp_ps, EXP,
                            bias=stab_sb, scale=1.0)

              for s in range(NSL):
                    # ---- q features for slice s ----
                    for mc in range(3):
                        mrows = 128 if mc < 2 else 10
                        qp_ps = psq.tile([128, SL], F32, tag="qp")
                        nc.tensor.matmul(
                            qp_ps[0:mrows, :], rft_r[:, ds(mc * 128, mrows)],
                            qhr[:, ts(s, SL)], start=True, stop=True,
                            skip_group_check=True)
                        nc.scalar.activation(
                            qt_feat[0:mrows, mc, ts(s, SL)],
                            qp_ps[0:mrows, :], EXP, bias=0.0, scale=1.0)

                    # ---- k features + scan for the 4 chunks of slice s ----
                    for ch in range(4 * s, 4 * s + 4):
                        if ch == 0:
                            emit_kp(0)
                        if ch + 1 < NCH:
                            emit_kp(ch + 1)
                        # transpose k~ chunk to m-major
                        tp_ps = pstr.tile([128, 3, 128], BF16, tag="tp")
                        nc.tensor.transpose(tp_ps[:, 0, :], ktr[:, ch, 0:128], id_r)
                        nc.tensor.transpose(tp_ps[:, 1, :], ktr[:, ch, 128:256], id_r)
                        nc.tensor.transpose(tp_ps[0:10, 2, :], ktr[:, ch, 256:266], id_r)
                        ktT = ktTp.tile([128, 3, C], BF16, tag="ktT", name=f"ktT{ch}")
                        (nc.scalar.copy if ch % 2 == 0 else
                         nc.vector.tensor_copy)(ktT[:, 0, :], tp_ps[:, 0, :])
                        # rows 10.. of the mc2 block are never read; copying
                        # them (uninitialized) is harmless and merges two
                        # copies into one
                        nc.vector.tensor_copy(ktT[:, 1:3, :], tp_ps[:, 1:3, :])
                        ktT_tiles[ch] = ktT

                        # diagonal score block (keys ch x queries ch), masked
                        at_ps = psat.tile([C, C], F32, tag="at")
                        for mc in range(2):
                            nc.tensor.matmul(at_ps, ktT[:, mc, :],
                                             qt_feat[:, mc, ts(ch, C)],
                                             start=(mc == 0), stop=False,
                                             skip_group_check=True)
                        nc.tensor.matmul(at_ps, ktT[0:10, 2, :],
                                         qt_feat[0:10, 2, ts(ch, C)],
                                         start=False, stop=True,
                                         skip_group_check=True)
                        at_r = atp.tile([C, C], BF16, tag="at_r", name=f"atr{ch}")
                        nc.vector.tensor_mul(at_r, at_ps, tri_r)
                        at_list.append(at_r)

                        # state update for this chunk
                        sd_ps = pssd.tile([128, 3, 66], F32, tag="sd")
                        if ch == 0:
                            # one-time zero of mc2 rows the matmuls never
                            # write; persists across psum reuse (bufs=1)
                            nc.vector.memset(sd_ps[:, 2, :], 0.0)
                        for mc, cols in ((0, (0, 128)), (1, (128, 256)),
                                         (2, (256, 266))):
                            dst = sd_ps[:, mc, :] if mc < 2 else sd_ps[0:10, 2, :]
                            nc.tensor.matmul(dst, ktr[:, ch, cols[0]:cols[1]],
                                             vha[:, ch, 0:66], start=True,
                                             stop=True, skip_group_check=True)
                        if ch == 0:
                            nc.vector.tensor_copy(s_tiles[1][:, :, :], sd_ps)
                        else:
                            nc.vector.tensor_add(s_tiles[ch + 1][:, :, :],
                                                 s_tiles[ch][:, :, :], sd_ps)

            # z column = final state's column 64
            zcol = feat.tile([128, 3, 1], BF16)
            nc.vector.tensor_copy(zcol, s_tiles[NCH][:, :, 64:65])

            # ---- per-chunk output, normalize, FC (software-pipelined:
            # the DVE/Pool normalization chain for chunk ch runs while the
            # PE issues o-matmuls for ch+1/ch+2, then transpose+FC trail
            # two chunks behind) ----
            with (
                tc.tile_pool(name="pso", bufs=4, space="PSUM") as pso,
                tc.tile_pool(name="psfc", bufs=2, space="PSUM") as psfc,
                tc.tile_pool(name="pstr2", bufs=2, space="PSUM") as pstr2,
            ):
                o_tiles = {}
                rec_tiles = {}
                for it in range(NCH + 3):
                    if it >= 3:
                        ch = it - 3
                        o_ps = o_tiles.pop(ch)
                        attn_r = post.tile([C, D_V], F32, tag="attn",
                                           name=f"at{ch}")
                        nc.vector.tensor_scalar_mul(attn_r, o_ps[:, 0:D_V],
                                                    rec_tiles.pop(ch))
                        tr_ps = pstr2.tile([D_V, C], F32, tag="tr")
                        nc.tensor.transpose(tr_ps, attn_r, idf_r)
                        attnT_r = post.tile([D_V, C], BF16, tag="attnT",
                                            name=f"aT{ch}")
                        nc.scalar.copy(attnT_r, tr_ps)
                        fc_ps = psfc.tile([C, D_MODEL], F32, tag="fc")
                        nc.tensor.matmul(fc_ps, attnT_r, wfc_r, start=True,
                                         stop=True, skip_group_check=True)
                        o_sb = outp.tile([C, D_MODEL], BF16, tag="osb",
                                         name=f"osb{ch}")
                        eng = (nc.scalar.copy, nc.vector.tensor_copy)[ch % 2]
                        eng(o_sb, fc_ps)
                        nc.sync.dma_start(out=out_d[ts(ch, C), :], in_=o_sb)
                    if it < NCH:
                        ch = it
                        o_ps = pso.tile([C, 67], F32, tag="o", name=f"o{ch}")
                        if ch > 0:
                            for mc in range(2):
                                nc.tensor.matmul(o_ps[:, 0:66],
                                                 qt_feat[:, mc, ts(ch, C)],
                                                 s_tiles[ch][:, mc, 0:66],
                                                 start=(mc == 0), stop=False,
                                                 skip_group_check=True)
                            nc.tensor.matmul(o_ps[:, 0:66],
                                             qt_feat[0:10, 2, ts(ch, C)],
                                             s_tiles[ch][0:10, 2, 0:66],
                                             start=False, stop=False,
                                             skip_group_check=True)
                            nc.tensor.matmul(o_ps[:, 0:66], at_list[ch],
                                             vha[:, ch, :], start=False,
                                             stop=True, skip_group_check=True)
                        else:
                            nc.tensor.matmul(o_ps[:, 0:66], at_list[ch],
                                             vha[:, ch, :], start=True,
                                             stop=True, skip_group_check=True)
                        # d = q~ . z in column 66
                        for mc in range(2):
                            nc.tensor.matmul(o_ps[:, 66:67],
                                             qt_feat[:, mc, ts(ch, C)],
                                             zcol[:, mc, :], start=(mc == 0),
                                             stop=False, skip_group_check=True)
                        nc.tensor.matmul(o_ps[:, 66:67],
                                         qt_feat[0:10, 2, ts(ch, C)],
                                         zcol[0:10, 2, :], start=False,
                                         stop=True, skip_group_check=True)
                        o_tiles[ch] = o_ps
                        dcols = post.tile([C, 3], F32, tag="dcols",
                                          name=f"dc{ch}")
                        nc.vector.tensor_copy(dcols, o_ps[:, 64:67])
                        dd = post.tile([C, 1], F32, tag="dd", name=f"dd{ch}")
                        nc.vector.tensor_mul(dd, dcols[:, 0:1], dcols[:, 2:3])
                        rec = post.tile([C, 1], F32, tag="rec", name=f"rc{ch}")
                        nc.vector.reciprocal(rec, dd)
                        rec_tiles[ch] = rec
    nc.compile()
    return nc


# --------------------------------------------------------------------------
# Host orchestration
# --------------------------------------------------------------------------
_CACHE = {}


def _get_programs():
    if "a" not in _CACHE:
        _CACHE["a"] = build_phase_a()
        _CACHE["b"] = build_phase_b()
    return _CACHE["a"], _CACHE["b"]


def _bf(x):
    return np.ascontiguousarray(np.asarray(x, np.float32).astype(NPBF))


def _prep_a_maps(q, k, v, Wq, Wk, Wv, gamma, beta):
    qT = np.ascontiguousarray(q[0].T)
    kT = np.ascontiguousarray(k[0].T)
    vT = np.ascontiguousarray(v[0].T)
    Wqe = (gamma[:, None] * Wq) * SCALE
    Wke = Wk * SCALE
    cq_all = (beta @ Wq) * SCALE                       # [512]
    cqm = np.zeros((128, 5), np.float32)
    cqm[:, 0:4] = cq_all.reshape(4, 128).T
    cqm[0, 4] = LN_EPS
    cba = np.zeros((128, 4 + D_MODEL + 128), np.float32)
    cba[:, 0] = 1.0 / D_MODEL
    cba[:, 1] = 1.0
    cba[0:64, 2] = -0.5
    cba[64:128, 3] = -0.5
    cba[0, 4:4 + D_MODEL] = -Wqe.sum(axis=0)
    cba[0, 4 + D_MODEL:] = 1.0
    Wqv = np.concatenate([Wqe, Wv], axis=1)
    Wqv_b, Wke_b, cba_b = (_bf(Wqv), _bf(Wke), _bf(cba))
    in_a = []
    for j in range(NC):
        sl = slice(j * SLA, (j + 1) * SLA)
        xs = np.concatenate([qT[:, sl], kT[:, sl], vT[:, sl]], axis=1)
        in_a.append({
            "xs": _bf(xs),
            "Wke": Wke_b, "Wqv": Wqv_b,
            "cba": cba_b, "cqm": cqm,
        })
    return in_a


def _prep_b_maps(W_fc, rf, res_a):
    kh_full = np.concatenate([np.asarray(r["kh"]) for r in res_a], axis=1)
    qh_full = np.concatenate([np.asarray(r["qh"]) for r in res_a], axis=1)
    vh_full = np.concatenate([np.asarray(r["vhT"]) for r in res_a], axis=0)
    k_stab = np.float32(max(float(np.max(r["stab"])) for r in res_a))

    rftT = rf.T.astype(np.float32)                     # [64, 266]
    tri = np.triu(np.ones((C, C), np.float32))
    identm = np.eye(128, dtype=np.float32)
    stabc = np.concatenate([np.full((128, 1), -k_stab, np.float32),
                            identm], axis=1)

    in_b = []
    for h in range(NC):
        rows = slice(h * D_K, (h + 1) * D_K)
        vh_h = np.asarray(vh_full[:, h * D_V:(h + 1) * D_V], np.float32)
        vht = vh_h.reshape(NCH, 128, D_V).transpose(1, 0, 2).reshape(
            128, NCH * D_V)
        blob1 = np.concatenate([rftT, np.full((D_K, M), -0.5, np.float32)],
                               axis=1)
        blob2 = np.zeros((128, 128 + 128 + D_MODEL), np.float32)
        blob2[:, 0:128] = identm
        blob2[:, 128:256] = tri
        blob2[0:D_K, 256:256 + D_MODEL] = W_fc[rows, :] * float(M)
        in_b.append({
            "khh": _bf(kh_full[rows]),
            "qhh": _bf(qh_full[rows]),
            "vht": _bf(vht),
            "blob1": _bf(blob1),
            "blob2": _bf(blob2),
            "stabc": stabc,
        })
    return in_b


def _cast_all(*arrs):
    return [np.asarray(a, np.float32) for a in arrs]


def kernel(q, k, v, Wq, Wk, Wv, W_fc, b_fc, gamma, beta, rf):
    q, k, v, Wq, Wk, Wv, W_fc, b_fc, gamma, beta, rf = _cast_all(
        q, k, v, Wq, Wk, Wv, W_fc, b_fc, gamma, beta, rf)

    nc_a, nc_b = _get_programs()
    cores = list(range(NC))

    in_a = _prep_a_maps(q, k, v, Wq, Wk, Wv, gamma, beta)
    res_a = run_bass_kernel_spmd(nc_a, in_a, core_ids=cores)

    in_b = _prep_b_maps(W_fc, rf, res_a.results)
    res_b = run_bass_kernel_spmd(nc_b, in_b, core_ids=cores)

    out = np.zeros((N, D_MODEL), np.float32)
    for r in res_b.results:
        out += np.asarray(r["out"], np.float32)
    out += b_fc[None, :]
    out += q[0]
    return out[None].astype(np.float32)


def trace_args(inputs):
    """For test.py: returns [(phase, nc, in_maps), ...] re-runnable with trace."""
    q, k, v, Wq, Wk, Wv, W_fc, b_fc, gamma, beta, rf = _cast_all(
        inputs["q"], inputs["k"], inputs["v"], inputs["Wq"], inputs["Wk"],
        inputs["Wv"], inputs["W_fc"], inputs["b_fc"], inputs["gamma"],
        inputs["beta"], inputs["rf"])
    nc_a, nc_b = _get_programs()
    in_a = _prep_a_maps(q, k, v, Wq, Wk, Wv, gamma, beta)
    res_a = run_bass_kernel_spmd(nc_a, in_a, core_ids=list(range(NC)))
    in_b = _prep_b_maps(W_fc, rf, res_a.results)
    return [("a", nc_a, in_a), ("b", nc_b, in_b)]


if __name__ == "__main__":
    rng = np.random.default_rng(0)
    inputs = {
        "q": rng.standard_normal((1, N, D_MODEL)).astype(np.float32),
        "k": rng.standard_normal((1, N, D_MODEL)).astype(np.float32),
        "v": rng.standard_normal((1, N, D_MODEL)).astype(np.float32),
        "Wq": (rng.standard_normal((D_MODEL, 512)) * 0.04).astype(np.float32),
        "Wk": (rng.standard_normal((D_MODEL, 512)) * 0.04).astype(np.float32),
        "Wv": (rng.standard_normal((D_MODEL, 512)) * 0.04).astype(np.float32),
        "W_fc": (rng.standard_normal((512, D_MODEL)) * 0.04).astype(np.float32),
        "b_fc": np.zeros(D_MODEL, np.float32),
        "gamma": np.ones(D_MODEL, np.float32),
        "beta": np.zeros(D_MODEL, np.float32),
        "rf": rng.standard_normal((M, D_K)).astype(np.float32),
    }
    out = kernel(**inputs)
    print("kernel output", out.shape, out.dtype)
